# revision 1
# baseline (speedup 1.0000x reference)
"""HaloAttention Trainium2 kernel — 8 NeuronCores, data-parallel over (batch x 4-block-row strips).

Per-core strip: x[b, :, 32s-3 : 32s+35, :] zero-padded -> [256, 38, 134].
Pipeline per core:
  1. q~T / k~T channel-major projections (heads zero-padded to 32 rows for 32-aligned
     partition bases), x-patch V projections in pixel-major interleaved layout.
  2. Per block (64): QK per head-pair -> S [128, 196] PSUM; ACT exp (scale folded,
     accum_out = softmax denominator); normalize E by 1/den; PE-transpose -> A^T;
     AV (lhsT = V-interleaved [kpix, 32], rhs = A^T) -> O^T gappy PSUM [4h x 32, 64].
  3. Proj with gap-padded weights batched over 8 blocks (N=512), bias, DMA out.
"""

import os
import sys
from contextlib import ExitStack

import numpy as np
import ml_dtypes

if "/opt/trn_rl_repo" not in sys.path:
    sys.path.insert(0, "/opt/trn_rl_repo")

import concourse.bass as bass
import concourse.tile as tile
from concourse import bacc
from concourse import mybir
from concourse.masks import make_identity
from concourse.bass_utils import run_bass_kernel_spmd

BF16 = mybir.dt.bfloat16
F32 = mybir.dt.float32

C = 256
HEADS = 16
HD = 16
HALO = 3
PATCH = 14
H = W = 128
B = 2
SCALE = HD ** -0.5

SR = 4                      # block rows per core
HS = SR * 8 + 2 * HALO      # 38
WS = W + 2 * HALO           # 134
NPIX = HS * WS              # 5092
NINT = SR * 8 * W           # 4096
NBC = 16
NB = SR * NBC               # 64
KA, KB = 126, 70            # patch chunks: rows 0..8 (9*14), rows 9..13 (5*14)

_CACHED = {}


def build_kernel(phases=15, reps=1):
    nc = bacc.Bacc("TRN2", target_bir_lowering=False, debug=False,
                   enable_asserts=False, num_devices=8)

    xs_d = nc.dram_tensor("xs", [2, 128, NPIX], BF16, kind="ExternalInput")
    wqk_d = nc.dram_tensor("wqk", [2, 128, 1024], BF16, kind="ExternalInput")
    wv_d = nc.dram_tensor("wv", [2, 128, 288], BF16, kind="ExternalInput")
    wp_d = nc.dram_tensor("wp", [4, 128, 256], BF16, kind="ExternalInput")
    bp_d = nc.dram_tensor("bp", [128, 2], F32, kind="ExternalInput")
    o_d = nc.dram_tensor("o", [2, 128, 4096], F32, kind="ExternalOutput")

    with tile.TileContext(nc) as tc, ExitStack() as ctx:
        consts = ctx.enter_context(tc.tile_pool(name="consts", bufs=1))
        bigbuf = ctx.enter_context(tc.tile_pool(name="bigbuf", bufs=1))

        ident = consts.tile([128, 128], BF16)
        make_identity(nc, ident)

        xT = [bigbuf.tile([128, NPIX], BF16, tag=f"xT{c}", name=f"xT{c}") for c in range(2)]
        wqk = [bigbuf.tile([128, 1024], BF16, tag=f"wqk{c}", name=f"wqk{c}") for c in range(2)]
        wv = [bigbuf.tile([128, 288], BF16, tag=f"wv{c}", name=f"wv{c}") for c in range(2)]
        wp = [bigbuf.tile([128, 256], BF16, tag=f"wp{k}", name=f"wp{k}") for k in range(4)]
        bp = consts.tile([128, 2], F32)

        for c in range(2):
            nc.sync.dma_start(xT[c][:], xs_d[c])
            nc.sync.dma_start(wqk[c][:], wqk_d[c])
            nc.sync.dma_start(wv[c][:], wv_d[c])
        for k in range(4):
            nc.sync.dma_start(wp[k][:], wp_d[k])
        nc.sync.dma_start(bp[:], bp_d[:])

        qT = [bigbuf.tile([128, NINT], BF16, tag=f"qT{m}", name=f"qT{m}") for m in range(4)]
        kT = [bigbuf.tile([128, NPIX], BF16, tag=f"kT{m}", name=f"kT{m}") for m in range(4)]

        xT3 = [t[:].rearrange("p (a b) -> p a b", b=WS) for t in xT]     # [128,38,134]
        kT3 = [t[:].rearrange("p (a b) -> p a b", b=WS) for t in kT]

        # ---------------- phase 1: q~ / k~ projections ----------------
        # qT stored block-contiguous: col = 64*blk + 8*qr + qc so QK lhsT is a
        # contiguous 64-col slice (matmul weights APs allow only one free dim).
        qT5 = [t[:].rearrange("p (br cb qr qc) -> p br qr cb qc", br=4, cb=16, qr=8)
               for t in qT]
        with tc.tile_pool(name="qkv_ps", bufs=4, space="PSUM") as qkv_ps:
            for m in range(4):  # q~ chunks (interior pixels, 8 tiles of 4 rows)
                for t in range(8):
                    ps = qkv_ps.tile([128, 512], F32, tag="ps", name="ps")
                    for cc in range(2):
                        rhs = xT3[cc][:, HALO + 4 * t:HALO + 4 * t + 4, HALO:HALO + W]
                        nc.tensor.matmul(ps[:], wqk[cc][:, 128 * m:128 * (m + 1)],
                                         rhs, start=(cc == 0), stop=(cc == 1))
                    br, qr0 = (4 * t) // 8, (4 * t) % 8
                    dst = qT5[m][:, br, qr0:qr0 + 4]
                    nc.any.tensor_copy(out=dst,
                                       in_=ps[:].rearrange("p (a b c) -> p a b c",
                                                           a=4, b=16))
            for m in range(4):  # k~ chunks (all strip pixels)
                for t in range(10):
                    n = min(512, NPIX - 512 * t)
                    ps = qkv_ps.tile([128, 512], F32, tag="ps", name="ps")
                    for cc in range(2):
                        nc.tensor.matmul(ps[:, :n],
                                         wqk[cc][:, 128 * (4 + m):128 * (5 + m)],
                                         xT[cc][:, 512 * t:512 * t + n],
                                         start=(cc == 0), stop=(cc == 1))
                    nc.any.tensor_copy(out=kT[m][:, 512 * t:512 * t + n],
                                       in_=ps[:, :n])

        # ---------------- phase 2: attention ----------------
        with tc.tile_pool(name="s_ps", bufs=2, space="PSUM") as s_ps_pool, \
             tc.tile_pool(name="at_ps", bufs=2, space="PSUM") as at_ps_pool, \
             tc.tile_pool(name="vp_ps", bufs=1, space="PSUM") as vp_ps_pool, \
             tc.tile_pool(name="gap_ps", bufs=1, space="PSUM") as gap_ps_pool, \
             tc.tile_pool(name="op_ps", bufs=1, space="PSUM") as op_ps_pool, \
             tc.tile_pool(name="work", bufs=3) as work, \
             tc.tile_pool(name="epool", bufs=10) as epool, \
             tc.tile_pool(name="atpool", bufs=3) as atpool, \
             tc.tile_pool(name="gpool", bufs=2) as gpool:

            gap_sb = None
            for _rep in range(reps):
              for blk in range(NB):
                  r, cb = blk // NBC, blk % NBC
                  g = cb % 8
                  if g == 0:
                      gap_sb = gpool.tile([128, 4, 512], BF16, tag="gap_sb", name="gap_sb")

                  # --- V patch (interleaved 17 cols/head) ---
                  # im2col the x patch (SBUF->SBUF DMA) so the matmul stationary
                  # operand has a single contiguous free dim
                  xp_sb = work.tile([128, 2, 196], BF16, tag="xp", name="xp")
                  for cc in range(2):
                      nc.gpsimd.tensor_copy(
                          out=xp_sb[:, cc, :].rearrange("p (a b) -> p a b", a=PATCH),
                          in_=xT3[cc][:, 8 * r:8 * r + PATCH, 8 * cb:8 * cb + PATCH])
                  vp_ps_a = vp_ps_pool.tile([128, 288], F32, tag="vpa", name="vpa")
                  vp_ps_b = vp_ps_pool.tile([128, 288], F32, tag="vpb", name="vpb")
                  for cc in range(2):
                      nc.tensor.matmul(vp_ps_a[:KA, :], xp_sb[:, cc, :KA], wv[cc][:],
                                       start=(cc == 0), stop=(cc == 1))
                      nc.tensor.matmul(vp_ps_b[:KB, :], xp_sb[:, cc, KA:196], wv[cc][:],
                                       start=(cc == 0), stop=(cc == 1))
                  vp_a = work.tile([128, 288], BF16, tag="vpa_sb", name="vpa_sb")
                  vp_b = work.tile([128, 288], BF16, tag="vpb_sb", name="vpb_sb")
                  nc.any.tensor_copy(out=vp_a[:KA], in_=vp_ps_a[:KA])
                  nc.any.tensor_copy(out=vp_b[:KB], in_=vp_ps_b[:KB])

                  den = work.tile([128, 8], F32, tag="den", name="den")
                  rden = work.tile([128, 8], F32, tag="rden", name="rden")
                  e_tiles = []
                  for p in (range(8) if phases & 1 else []):
                      s_ps = s_ps_pool.tile([128, 196], F32, tag="s", name="s")
                      for i, hh in enumerate((2 * p, 2 * p + 1)):
                          mc, st = hh // 4, 32 * (hh % 4)
                          lq = qT[mc][st:st + 32, 64 * blk:64 * blk + 64]
                          rk = kT3[mc][st:st + 32, 8 * r:8 * r + PATCH,
                                       8 * cb:8 * cb + PATCH]
                          nc.tensor.matmul(s_ps[64 * i:64 * (i + 1), :], lq, rk,
                                           tile_position=(st, 64 * i))
                      e_sb = epool.tile([128, 196], BF16, tag="e", name="e")
                      nc.scalar.activation(e_sb[:], s_ps[:],
                                           mybir.ActivationFunctionType.Exp,
                                           scale=SCALE, accum_out=den[:, p:p + 1])
                      e_tiles.append(e_sb)

                  if phases & 1:
                      nc.vector.reciprocal(rden[:], den[:])

                  gap_ps = gap_ps_pool.tile([128, 4, 64], F32, tag="gap", name="gap")
                  for p in (range(8) if phases & 2 else []):
                      e_sb = e_tiles[p]
                      nc.gpsimd.tensor_scalar_mul(e_sb[:], e_sb[:], rden[:, p:p + 1])
                      at_ps = at_ps_pool.tile([128, 2, 128], BF16, tag="at", name="at")
                      nc.tensor.transpose(at_ps[:KA, 0, :], e_sb[:, :KA], ident[:])
                      nc.tensor.transpose(at_ps[:KB, 1, :], e_sb[:, KA:196], ident[:])
                      at_sb = atpool.tile([128, 2, 128], BF16, tag="at_sb", name="at_sb")
                      nc.any.tensor_copy(out=at_sb[:], in_=at_ps[:])

                      for i, hh in (enumerate((2 * p, 2 * p + 1)) if phases & 4 else []):
                          outp = gap_ps[32 * (hh % 4):32 * (hh % 4) + 32, hh // 4, :]
                          tp = (0, 32 * (hh % 4))
                          nc.tensor.matmul(outp, vp_a[:KA, 17 * hh:17 * hh + 32],
                                           at_sb[:KA, 0, 64 * i:64 * (i + 1)],
                                           start=True, stop=False, tile_position=tp)
                          nc.tensor.matmul(outp, vp_b[:KB, 17 * hh:17 * hh + 32],
                                           at_sb[:KB, 1, 64 * i:64 * (i + 1)],
                                           start=False, stop=True, tile_position=tp)

                  gs5 = gap_sb[:].rearrange("p k (a b c) -> p k a b c", a=8, b=8)
                  if not (phases & 4):
                      nc.vector.memset(gap_ps[:, :, :], 0.0)
                  nc.any.tensor_copy(
                      out=gs5[:, :, :, g, :],
                      in_=gap_ps[:].rearrange("p k (a c) -> p k a c", a=8))

                  if g == 7 and phases & 8:
                      half = (cb // 8)
                      for mc in range(2):
                          op_ps = op_ps_pool.tile([128, 512], F32, tag="op", name="op")
                          for kc in range(4):
                              nc.tensor.matmul(op_ps[:],
                                               wp[kc][:, 128 * mc:128 * (mc + 1)],
                                               gap_sb[:, kc, :],
                                               start=(kc == 0), stop=(kc == 3))
                          out_sb = work.tile([128, 512], F32, tag="out_sb", name="out_sb")
                          nc.vector.tensor_scalar_add(out_sb[:], op_ps[:],
                                                      bp[:, mc:mc + 1])
                          # out_sb cols are (qr 8, b'' 8, qc 8); dst rows qr,
                          # block-cols half*64 .. half*64+64 contiguous
                          o3 = o_d[mc].rearrange("p (row col) -> p row col", col=W)
                          nc.sync.dma_start(
                              o3[:, 8 * r:8 * r + 8, 64 * half:64 * half + 64],
                              out_sb[:].rearrange("p (a bc) -> p a bc", a=8))

    nc.compile()
    return nc


def _prep_host(x, w_qkv, w_proj, b_proj):
    bf = ml_dtypes.bfloat16
    xp = np.pad(np.asarray(x, np.float32),
                ((0, 0), (0, 0), (HALO, HALO), (HALO, HALO)))
    wq, wk, wvv = w_qkv[:C], w_qkv[C:2 * C], w_qkv[2 * C:]

    wqk_pad = np.zeros((1024, C), np.float32)
    for h in range(HEADS):
        wqk_pad[32 * h:32 * h + HD] = wq[HD * h:HD * (h + 1)]
        wqk_pad[512 + 32 * h:512 + 32 * h + HD] = wk[HD * h:HD * (h + 1)]
    wqkT = np.ascontiguousarray(wqk_pad.T).astype(bf).reshape(2, 128, 1024)

    wv_int = np.zeros((288, C), np.float32)
    for h in range(HEADS):
        wv_int[17 * h:17 * h + HD] = wvv[HD * h:HD * (h + 1)]
    wvT = np.ascontiguousarray(wv_int.T).astype(bf).reshape(2, 128, 288)

    wp_gap = np.zeros((512, C), np.float32)
    for h in range(HEADS):
        wp_gap[32 * h:32 * h + HD] = w_proj[:, HD * h:HD * (h + 1)].T
    wpT = np.ascontiguousarray(wp_gap).astype(bf).reshape(4, 128, 256)

    bpT = np.ascontiguousarray(np.asarray(b_proj, np.float32).reshape(2, 128).T)

    in_maps = []
    for core in range(8):
        b, s = core // 4, core % 4
        strip = xp[b, :, 32 * s:32 * s + HS, :]
        xs = np.ascontiguousarray(strip.reshape(2, 128, NPIX)).astype(bf)
        in_maps.append({"xs": xs, "wqk": wqkT, "wv": wvT, "wp": wpT, "bp": bpT})
    return in_maps


def kernel(x, w_qkv, w_proj, b_proj):
    if "nc" not in _CACHED:
        _CACHED["nc"] = build_kernel()
    nc = _CACHED["nc"]
    in_maps = _prep_host(np.asarray(x), np.asarray(w_qkv),
                         np.asarray(w_proj), np.asarray(b_proj))
    res = run_bass_kernel_spmd(nc, in_maps, core_ids=list(range(8)))
    _CACHED["last_results"] = res
    out = np.zeros((B, C, H, W), np.float32)
    for core in range(8):
        b, s = core // 4, core % 4
        o = np.concatenate([res.results[core]["o"][0], res.results[core]["o"][1]],
                           axis=0).reshape(C, 32, W)
        out[b, :, 32 * s:32 * s + 32, :] = o
    return out



# revision 22
# speedup vs baseline: 2.0644x; 2.0644x over previous
"""HaloAttention Trainium2 kernel — 8 NeuronCores, data-parallel over (batch x 4-block-row strips).

Per-core strip: x[b, :, 32s-3 : 32s+35, :] zero-padded -> [256, 38, 134].
v2 pipeline (S^T orientation, deferred softmax normalization):
  1. q~T / k~T channel-major projections (heads zero-padded to 32 rows), as before.
  2. Per block (64):
     - im2col k~ patch (kp, 4 m-chunks) and x patch (xp) via strided copies.
     - V projection from xp -> vp psum -> SBUF (data cols only); spare col 16 of
       each 17-col head group holds 1.0 so the AV matmul also produces the
       softmax denominator rows for free.
     - S^T = kp^T q~ per head into one 4-bank PSUM tile, bank-cycled so the four
       concurrent 32x128 PE row-tiles hit disjoint banks.
     - ONE exp activation [128, 2048] (scale folded), no accumulation.
     - AV: O^T += vp^T E chunks into gap psum quadrants; row 32g+16 = den.
     - Deferred normalize: reciprocal(gap) -> sel-matmul broadcasts 1/den to all
       rows -> tensor_tensor multiply -> gap_sb slot.
  3. Proj with gap-padded weights batched over 8 blocks (N=512), bias fused into
     an Identity activation, contiguous bf16 DMA out.
"""

import sys

import numpy as np
import ml_dtypes

if "/opt/trn_rl_repo" not in sys.path:
    sys.path.insert(0, "/opt/trn_rl_repo")

import concourse.bass as bass
import concourse.tile as tile
from concourse import bacc
from concourse import mybir
from concourse.bass_utils import run_bass_kernel_spmd

BF16 = mybir.dt.bfloat16
F32 = mybir.dt.float32

C = 256
HEADS = 16
HD = 16
HALO = 3
PATCH = 14
H = W = 128
B = 2
SCALE = HD ** -0.5

SR = 4                      # block rows per core
HS = SR * 8 + 2 * HALO      # 38
WS = W + 2 * HALO           # 134
NPIX = HS * WS              # 5092
NINT = SR * 8 * W           # 4096
NBC = 16
NB = SR * NBC               # 64
KA, KB = 128, 68            # key chunks (196 = 128 + 68)

_CACHED = {}


def build_kernel():
    nc = bacc.Bacc("TRN2", target_bir_lowering=False, debug=False,
                   enable_asserts=False, num_devices=8)

    xs_d = nc.dram_tensor("xs", [2, 128, NPIX], BF16, kind="ExternalInput")
    wqk_d = nc.dram_tensor("wqk", [2, 128, 1024], BF16, kind="ExternalInput")
    wv_d = nc.dram_tensor("wv", [2, 128, 256], BF16, kind="ExternalInput")
    wp_d = nc.dram_tensor("wp", [4, 128, 256], BF16, kind="ExternalInput")
    bp_d = nc.dram_tensor("bp", [128, 2], F32, kind="ExternalInput")
    sel_d = nc.dram_tensor("sel", [128, 128], BF16, kind="ExternalInput")
    o_d = nc.dram_tensor("o", [2, 128, 4096], BF16, kind="ExternalOutput")

    with tile.TileContext(nc) as tc:
      with tc.tile_pool(name="consts", bufs=1) as consts, \
           tc.tile_pool(name="bigbuf", bufs=1) as bigbuf:

        xT = [bigbuf.tile([128, NPIX], BF16, tag=f"xT{c}", name=f"xT{c}") for c in range(2)]
        wqk = [bigbuf.tile([128, 1024], BF16, tag=f"wqk{c}", name=f"wqk{c}") for c in range(2)]
        wv = [bigbuf.tile([128, 256], BF16, tag=f"wv{c}", name=f"wv{c}") for c in range(2)]
        wp = [bigbuf.tile([128, 256], BF16, tag=f"wp{k}", name=f"wp{k}") for k in range(4)]
        bp = consts.tile([128, 2], F32)
        sel = consts.tile([128, 128], BF16)

        for c in range(2):
            nc.sync.dma_start(xT[c][:, 0:3072], xs_d[c][:, 0:3072])
        for c in range(2):
            nc.sync.dma_start(xT[c][:, 3072:NPIX], xs_d[c][:, 3072:NPIX])
        for c in range(2):
            nc.sync.dma_start(wqk[c][:], wqk_d[c])
            nc.sync.dma_start(wv[c][:], wv_d[c])
        for k in range(4):
            nc.sync.dma_start(wp[k][:], wp_d[k])
        nc.sync.dma_start(bp[:], bp_d[:])
        nc.sync.dma_start(sel[:], sel_d[:])

        qT = [bigbuf.tile([128, NINT], BF16, tag=f"qT{m}", name=f"qT{m}") for m in range(4)]
        kT = [bigbuf.tile([128, NPIX], BF16, tag=f"kT{m}", name=f"kT{m}") for m in range(4)]

        xT3 = [t[:].rearrange("p (a b) -> p a b", b=WS) for t in xT]     # [128,38,134]
        kT3 = [t[:].rearrange("p (a b) -> p a b", b=WS) for t in kT]

        # ---------------- phase 1: q~ / k~ projections ----------------
        # qT stored block-contiguous: col = 64*blk + 8*qr + qc so the S^T moving
        # operand is a contiguous 64-col slice.
        qT5 = [t[:].rearrange("p (br cb qr qc) -> p br qr cb qc", br=4, cb=16, qr=8)
               for t in qT]
        deferred_units = []

        def q_unit(qkv_ps, m, t, tag="ps"):
            ps = qkv_ps.tile([128, 512], F32, tag=tag, name=tag)
            for cc in range(2):
                rhs = xT3[cc][:, HALO + 4 * t:HALO + 4 * t + 4, HALO:HALO + W]
                nc.tensor.matmul(ps[:], wqk[cc][:, 128 * m:128 * (m + 1)],
                                 rhs, start=(cc == 0), stop=(cc == 1))
            br, qr0 = (4 * t) // 8, (4 * t) % 8
            dst = qT5[m][:, br, qr0:qr0 + 4]
            nc.any.tensor_copy(out=dst,
                               in_=ps[:].rearrange("p (a b c) -> p a b c",
                                                   a=4, b=16))

        def k_unit(qkv_ps, m, t, tag="ps"):
            n = min(512, NPIX - 512 * t)
            ps = qkv_ps.tile([128, 512], F32, tag=tag, name=tag)
            for cc in range(2):
                nc.tensor.matmul(ps[:, :n],
                                 wqk[cc][:, 128 * (4 + m):128 * (5 + m)],
                                 xT[cc][:, 512 * t:512 * t + n],
                                 start=(cc == 0), stop=(cc == 1))
            nc.any.tensor_copy(out=kT[m][:, 512 * t:512 * t + n],
                               in_=ps[:, :n])

        with tc.tile_pool(name="qkv_ps", bufs=4, space="PSUM") as qkv_ps:
            # A-half: k~ rows 0..24 (t 0..6) and q~ block-rows 0,1 (t 0..4),
            # enough for attention rows r=0,1. The rest interleaves with the
            # attention loop below (sharing the proj psum bank).
            for m in range(4):
                for t in range(4):
                    q_unit(qkv_ps, m, t)
            for m in range(4):
                for t in range(7):
                    k_unit(qkv_ps, m, t)
        for m in range(4):
            for t in range(4, 8):
                deferred_units.append(("q", m, t))
        for m in range(4):
            for t in range(7, 10):
                deferred_units.append(("k", m, t))

        # ---------------- phase 2: attention ----------------
        # PSUM bank map: s_ps 0-3 | vpa 4 | vpb 5 | gap+den 6 | proj 7
        with tc.tile_pool(name="s_psA", bufs=1, space="PSUM") as s_psA_pool, \
             tc.tile_pool(name="s_psB", bufs=1, space="PSUM") as s_psB_pool, \
             tc.tile_pool(name="vp_ps", bufs=1, space="PSUM") as vp_ps_pool, \
             tc.tile_pool(name="gap_ps", bufs=1, space="PSUM") as gap_ps_pool, \
             tc.tile_pool(name="den_ps", bufs=1, space="PSUM") as den_ps_pool, \
             tc.tile_pool(name="op_ps", bufs=1, space="PSUM") as op_ps_pool, \
             tc.tile_pool(name="kpool", bufs=2) as kpool, \
             tc.tile_pool(name="xpool", bufs=2) as xpool, \
             tc.tile_pool(name="vpool", bufs=2) as vpool, \
             tc.tile_pool(name="epool", bufs=2) as epool, \
             tc.tile_pool(name="rpool", bufs=2) as rpool, \
             tc.tile_pool(name="gpool", bufs=2) as gpool, \
             tc.tile_pool(name="opool", bufs=2) as opool:

            def copies_and_v(blk):
                r, cb = blk // NBC, blk % NBC
                # --- im2col: k~ patches (4 m-chunks) and x patch (2 chunks) ---
                kp = kpool.tile([128, 4, 256], BF16, tag="kp", name="kp")
                for mc in range(4):
                    nc.vector.tensor_copy(
                        out=kp[:, mc, 0:196].rearrange("p (a b) -> p a b", b=PATCH),
                        in_=kT3[mc][:, 8 * r:8 * r + PATCH, 8 * cb:8 * cb + PATCH])
                if blk < 2:  # zero the pad cols once per ring buffer
                    nc.vector.memset(kp[:, :, 196:256], 0.0)

                xp = xpool.tile([128, 2, 196], BF16, tag="xp", name="xp")
                for cc in range(2):
                    nc.gpsimd.tensor_copy(
                        out=xp[:, cc, :].rearrange("p (a b) -> p a b", a=PATCH),
                        in_=xT3[cc][:, 8 * r:8 * r + PATCH, 8 * cb:8 * cb + PATCH])

                # --- V projection (compact psum; SBUF gets 17-col interleave,
                # col 16 per group stays 1.0 for the denominator trick).
                # Close the A-chunk accumulation group before opening B's:
                # start=True clears has_written bits for the WHOLE bank.
                vp_ps = vp_ps_pool.tile([128, 512], F32, tag="vp", name="vp")
                for cc in range(2):
                    nc.tensor.matmul(vp_ps[0:KA, 0:256], xp[:, cc, 0:KA], wv[cc][:],
                                     start=(cc == 0), stop=(cc == 1))
                for cc in range(2):
                    nc.tensor.matmul(vp_ps[0:KB, 256:512], xp[:, cc, KA:196], wv[cc][:],
                                     start=(cc == 0), stop=(cc == 1))
                vpa = vpool.tile([128, 288], BF16, tag="vpa_sb", name="vpa_sb")
                vpb = vpool.tile([128, 288], BF16, tag="vpb_sb", name="vpb_sb")
                va3 = vpa[:, 0:272].rearrange("p (h c) -> p h c", c=17)
                vb3 = vpb[:, 0:272].rearrange("p (h c) -> p h c", c=17)
                nc.vector.tensor_copy(out=va3[:, :, 0:16],
                                      in_=vp_ps[:, 0:256].rearrange(
                                          "p (h c) -> p h c", c=16))
                nc.vector.tensor_copy(out=vb3[:, :, 0:16],
                                      in_=vp_ps[:, 256:512].rearrange(
                                          "p (h c) -> p h c", c=16))
                if blk < 2:  # ones cols (den trick) + defined pad, once per ring
                    nc.vector.memset(va3[:, :, 16], 1.0)
                    nc.vector.memset(vb3[:, :, 16], 1.0)
                    nc.vector.memset(vpa[:, 272:288], 0.0)
                    nc.vector.memset(vpb[:, 272:288], 0.0)
                return kp, vpa, vpb

            def emit_proj(gsb, grp):
                for mc in range(2):
                    op_ps = op_ps_pool.tile([128, 512], F32, tag="op", name="op")
                    for kc in range(4):
                        nc.tensor.matmul(op_ps[:],
                                         wp[kc][:, 128 * mc:128 * (mc + 1)],
                                         gsb[:, kc, :],
                                         start=(kc == 0), stop=(kc == 3))
                    out_sb = opool.tile([128, 512], BF16, tag="out_sb", name="out_sb")
                    nc.scalar.activation(out_sb[:], op_ps[:],
                                         mybir.ActivationFunctionType.Identity,
                                         bias=bp[:, mc:mc + 1])
                    nc.sync.dma_start(o_d[mc][:, 512 * grp:512 * grp + 512],
                                      out_sb[:])

            def finish_norm(graw, gslot, gsb, grp):
                den_ps = den_ps_pool.tile([128, 256], F32, tag="den", name="den")
                nc.tensor.matmul(den_ps[:], sel[:], graw[:], start=True, stop=True)
                rden = rpool.tile([128, 256], BF16, tag="rden", name="rden")
                with nc.allow_low_precision(reason="softmax denominator in bf16"):
                    nc.vector.reciprocal(rden[:], den_ps[:])
                gs5 = gsb[:].rearrange("p k (b q) -> p k b q", b=8)
                nc.gpsimd.tensor_tensor(
                    out=gs5[:, :, gslot, :],
                    in0=graw[:].rearrange("p (k c) -> p k c", k=4),
                    in1=rden[:].rearrange("p (k c) -> p k c", k=4),
                    op=mybir.AluOpType.mult)
                # group complete once slot 7's normalize lands
                return (gsb, grp) if gslot == 7 else None

            def emit_av(ctx):
                # AV for a previous block whose exp has long completed: O^T and
                # den accumulate into gap quadrants, then evacuate gap to SBUF.
                vpa, vpb, e4, gslot, gsb, grp = ctx
                gap_ps = gap_ps_pool.tile([128, 256], F32, tag="gap", name="gap")
                g4 = gap_ps[:].rearrange("p (k c) -> p k c", k=4)
                for h in range(HEADS):
                    gq, mc = h % 4, h // 4
                    outp = g4[32 * gq:32 * gq + 32, mc, :]
                    nc.tensor.matmul(outp, vpa[0:KA, 17 * h:17 * h + 32],
                                     e4[gq // 2][0:KA, gq % 2, 64 * mc:64 * mc + 64],
                                     start=True, stop=False,
                                     tile_position=(0, 32 * gq))
                    nc.tensor.matmul(
                        outp, vpb[0:KB, 17 * h:17 * h + 32],
                        e4[gq // 2][0:KB, gq % 2, 256 + 64 * mc:256 + 64 * mc + 64],
                        start=False, stop=True,
                        tile_position=(0, 32 * gq))
                graw = rpool.tile([128, 256], BF16, tag="graw", name="graw")
                nc.vector.tensor_copy(out=graw[:], in_=gap_ps[:])
                return graw, gslot, gsb, grp

            gap_sb = None
            pending_av = None     # (vpa, vpb, e4, gslot, gap_sb) of block n-1
            pending_norm = None   # (graw, gslot, gap_sb) of block n-2
            pending_proj = None
            pre = copies_and_v(0)
            for blk in range(NB):
                r, cb = blk // NBC, blk % NBC
                g = cb % 8
                kp, vpa, vpb = pre
                if g == 0:
                    gap_sb = gpool.tile([128, 4, 512], BF16, tag="gap_sb", name="gap_sb")

                # --- S^T in two halves (gq 0-1 -> banks 0-1, gq 2-3 -> 2-3) ---
                s_ps = [s_psA_pool.tile([128, 1024], F32, tag="sA", name="sA"),
                        s_psB_pool.tile([128, 1024], F32, tag="sB", name="sB")]
                s4 = [t[:].rearrange("p (g c) -> p g c", g=2) for t in s_ps]
                e_sb = [epool.tile([128, 1024], BF16, tag="eA", name="eA"),
                        epool.tile([128, 1024], BF16, tag="eB", name="eB")]
                e4 = [t[:].rearrange("p (g c) -> p g c", g=2) for t in e_sb]

                def st_half(hf):
                    for h in range(HEADS):
                        gq, mc = h % 4, h // 4
                        if gq // 2 != hf:
                            continue
                        lq = qT[mc][32 * gq:32 * gq + 32, 64 * blk:64 * blk + 64]
                        nc.tensor.matmul(s4[hf][:, gq % 2, 64 * mc:64 * mc + 64],
                                         kp[32 * gq:32 * gq + 32, mc, 0:128], lq,
                                         start=True, stop=True,
                                         tile_position=(32 * gq, 0))
                        nc.tensor.matmul(
                            s4[hf][:, gq % 2, 256 + 64 * mc:256 + 64 * mc + 64],
                            kp[32 * gq:32 * gq + 32, mc, 128:256], lq,
                            start=True, stop=True,
                            tile_position=(32 * gq, 0))

                st_half(0)
                nc.scalar.activation(e_sb[0][:], s_ps[0][:],
                                     mybir.ActivationFunctionType.Exp, scale=SCALE)
                st_half(1)
                nc.scalar.activation(e_sb[1][:], s_ps[1][:],
                                     mybir.ActivationFunctionType.Exp, scale=SCALE)

                # Everything below has all inputs ready (from earlier blocks), so
                # PE streams without waiting on this block's exp.
                old_proj = pending_proj
                pending_proj = None
                if pending_norm is not None:
                    maybe = finish_norm(*pending_norm)
                    pending_norm = None
                    if maybe is not None:
                        pending_proj = maybe
                if old_proj is not None:
                    emit_proj(*old_proj)
                if pending_av is not None:
                    pending_norm = emit_av(pending_av)
                    pending_av = None
                if blk + 1 < NB:
                    pre = copies_and_v(blk + 1)
                if deferred_units:
                    kind, m_, t_ = deferred_units.pop(0)
                    if kind == "q":
                        q_unit(op_ps_pool, m_, t_, tag="op")
                    else:
                        k_unit(op_ps_pool, m_, t_, tag="op")

                pending_av = (vpa, vpb, e4, g, gap_sb, 2 * r + (cb // 8))

            # drain the pipeline
            if pending_norm is not None:
                maybe = finish_norm(*pending_norm)
                assert maybe is None
            if pending_proj is not None:
                emit_proj(*pending_proj)
            pending_norm = emit_av(pending_av)
            maybe = finish_norm(*pending_norm)
            if maybe is not None:
                emit_proj(*maybe)

    nc.compile()
    return nc


def _prep_host(x, w_qkv, w_proj, b_proj):
    bf = ml_dtypes.bfloat16
    xp = np.pad(np.asarray(x, np.float32),
                ((0, 0), (0, 0), (HALO, HALO), (HALO, HALO)))
    wq, wk, wvv = w_qkv[:C], w_qkv[C:2 * C], w_qkv[2 * C:]

    wqk_pad = np.zeros((1024, C), np.float32)
    for h in range(HEADS):
        wqk_pad[32 * h:32 * h + HD] = wq[HD * h:HD * (h + 1)]
        wqk_pad[512 + 32 * h:512 + 32 * h + HD] = wk[HD * h:HD * (h + 1)]
    wqkT = np.ascontiguousarray(wqk_pad.T).astype(bf).reshape(2, 128, 1024)

    wvT = np.ascontiguousarray(wvv.T).astype(bf).reshape(2, 128, 256)

    wp_gap = np.zeros((512, C), np.float32)
    for h in range(HEADS):
        wp_gap[32 * h:32 * h + HD] = w_proj[:, HD * h:HD * (h + 1)].T
    wpT = np.ascontiguousarray(wp_gap).astype(bf).reshape(4, 128, 256)

    bpT = np.ascontiguousarray(np.asarray(b_proj, np.float32).reshape(2, 128).T)

    selm = np.zeros((128, 128), np.float32)
    for j in range(128):
        selm[32 * (j // 32) + 16, j] = 1.0
    selm = selm.astype(bf)

    in_maps = []
    for core in range(8):
        b, s = core // 4, core % 4
        strip = xp[b, :, 32 * s:32 * s + HS, :]
        xs = np.ascontiguousarray(strip.reshape(2, 128, NPIX)).astype(bf)
        in_maps.append({"xs": xs, "wqk": wqkT, "wv": wvT, "wp": wpT, "bp": bpT,
                       "sel": selm})
    return in_maps


def kernel(x, w_qkv, w_proj, b_proj):
    if "nc" not in _CACHED:
        _CACHED["nc"] = build_kernel()
    nc = _CACHED["nc"]
    in_maps = _prep_host(np.asarray(x), np.asarray(w_qkv),
                         np.asarray(w_proj), np.asarray(b_proj))
    res = run_bass_kernel_spmd(nc, in_maps, core_ids=list(range(8)))
    _CACHED["last_results"] = res
    out = np.zeros((B, C, H, W), np.float32)
    for core in range(8):
        b, s = core // 4, core % 4
        # o[mc] cols: (grp 8 = (r 4, half 2), b'' 8, qr 8, qc 8)
        o = np.concatenate([res.results[core]["o"][0], res.results[core]["o"][1]],
                           axis=0).astype(np.float32)
        o = o.reshape(C, 4, 2, 8, 8, 8).transpose(0, 1, 4, 2, 3, 5)
        out[b, :, 32 * s:32 * s + 32, :] = o.reshape(C, 32, W)
    return out
